# revision 25
# baseline (speedup 1.0000x reference)
"""Chamfer distance kernel for Trainium2 (Bass/Tile), SPMD over 8 NeuronCores.

Math (per batch b):
  dist[v,l] = ||x_v||^2 - 2 x_v.y_l + ||y_l||^2,  x=[1024,512], y=[512,512]
  out[b] = mean_v min_l dist + mean_l min_v dist

Strategy:
  - Data-parallel over batch: 64 batches -> 8 cores x 8 batches.
  - Host passes one tensor xy[b] = concat(-2*x^T, y^T) along the row dim
    ([D, Nv+Nl] per batch) so both matmul operands have the contraction
    dim D on partitions, loaded with one large DMA per batch.
  - Per batch on-chip (all matmuls in fp32r: fp32 storage, fast PE path):
      squares (one ACT pass) -> column-of-ones matmuls (PE) give
        a_row = 4*||x||^2 (rescaled 0.25 on the PSUM->SBUF copy) and
        b_row = ||y||^2 as [1, N] row vectors.
      main matmul (PE): pm[v-chunk] = sum_k (-2x)^T_k . y^T_k, plus one
        K=2 augmentation matmul adding a_v + b_l broadcast via
        stationary [ones; a] x moving [b; ones] => pm = full dist chunk.
      D1: free-dim min per chunk (DVE) -> [128,1] columns -> summed.
      D2: running elementwise min across chunks (DVE), then 4 PE
        transposes + free-dim mins to reduce across partitions.
  - Final: cross-partition sums via ones-matmul (exact fp32), scale,
    DMA out [1, 8] per core; host concatenates to [64].
"""

import numpy as np

N_CORES = 8
B = 8          # batches per core
D = 512        # feature dim
NV = 1024      # video clips
NL = 512       # language tokens
P = 128        # partitions
KC = D // P    # contraction chunks = 4
MC = NV // P   # v chunks = 8
NT = NV + NL   # combined x|y row length per k-chunk

IO_BUFS = 4

_CACHE = {}


def _build_bass():
    import concourse.bass as bass
    import concourse.mybir as mybir
    import concourse.tile as tile
    from concourse import bacc
    from concourse.masks import make_identity

    f32 = mybir.dt.float32
    f32r = mybir.dt.float32r
    ALU = mybir.AluOpType
    AX = mybir.AxisListType
    SQUARE = mybir.ActivationFunctionType.Square

    nc = bacc.Bacc(None)
    xy_h = nc.declare_dram_parameter("xy", [B, D, NT], f32r, isOutput=False)
    out_h = nc.declare_dram_parameter("out", [1, B], f32, isOutput=True)

    with tile.TileContext(nc) as tc:
        with (
            tc.tile_pool(name="const", bufs=1) as cpool,
            tc.tile_pool(name="io", bufs=IO_BUFS) as io,
            tc.tile_pool(name="work", bufs=2) as work,
            tc.tile_pool(name="acc", bufs=1) as accp,
            tc.tile_pool(name="ps", bufs=2, space="PSUM") as ps,
            tc.tile_pool(name="psn", bufs=1, space="PSUM") as psn,
        ):
            identity = cpool.tile([P, P], f32, tag="ident")
            make_identity(nc, identity)
            ones_f32 = cpool.tile([P, 1], f32, tag="onesf")
            nc.vector.memset(ones_f32, 1.0)
            # Memset can't write float32r (ISA check); produce f32r ones
            # via ACT copies, which round on write.
            ones_col = cpool.tile([P, 1], f32r, tag="ones")
            nc.scalar.copy(out=ones_col, in_=ones_f32)
            ones2_f32 = cpool.tile([2, NV], f32, tag="ones2f")
            nc.vector.memset(ones2_f32, 1.0)

            # Augmentation operands (double buffered by batch parity):
            #   aug_stat: partition 0 = ones, partition 1 = a_row
            #   aug_mov:  partition 0 = b_row, partition 1 = ones
            # K=2 contraction pairs 1*b_n + a_m*1. Engine writes must start
            # at a 32-aligned partition: ones rows come from the one-time
            # whole-tile copies, per-batch b lands on partition 0 via ACT,
            # per-batch a reaches partition 1 via a SBUF->SBUF DMA.
            aug_stat = [
                cpool.tile([2, NV], f32r, tag=f"augs{i}", name=f"aug_stat{i}")
                for i in range(2)
            ]
            aug_mov = [
                cpool.tile([2, NL], f32r, tag=f"augm{i}", name=f"aug_mov{i}")
                for i in range(2)
            ]
            for t in aug_stat:
                nc.scalar.copy(out=t, in_=ones2_f32)
            for t in aug_mov:
                nc.scalar.copy(out=t, in_=ones2_f32[:, :NL])

            d1sums = accp.tile([P, B], f32, tag="d1s")
            d2sums = accp.tile([P, B], f32, tag="d2s")
            dall = accp.tile([P, B], f32, tag="dall")
            out_sb = accp.tile([1, B], f32, tag="osb")

            def xsl(k, lo, hi):
                return slice(k * NT + lo, k * NT + hi)

            xytiles = {}

            def load_batch(b):
                xytiles[b] = io.tile([P, KC * NT], f32r, tag="xy", name=f"xyt{b}")
                nc.sync.dma_start(
                    out=xytiles[b],
                    in_=xy_h[b].rearrange("(k p) n -> p k n", p=P),
                )

            def norm_stage(b):
                """squares + norm matmuls + aug-operand copies for batch b
                (runs one batch ahead of the main stage)."""
                xytile = xytiles[b]
                sq = work.tile([P, KC * NT], f32r, tag="sq", name=f"sq{b}")
                nc.scalar.activation(out=sq, in_=xytile, func=SQUARE)
                a_ps = psn.tile([1, NV], f32, tag="aps", name=f"aps{b}")
                b_ps = psn.tile([1, NL], f32, tag="bps", name=f"bps{b}")
                for k in range(KC):
                    for h in range(2):
                        nc.tensor.matmul(
                            out=a_ps[:, h * 512 : (h + 1) * 512],
                            lhsT=ones_col,
                            rhs=sq[:, xsl(k, h * 512, (h + 1) * 512)],
                            start=(k == 0),
                            stop=(k == KC - 1),
                        )
                    nc.tensor.matmul(
                        out=b_ps,
                        lhsT=ones_col,
                        rhs=sq[:, xsl(k, NV, NT)],
                        start=(k == 0),
                        stop=(k == KC - 1),
                    )
                ast = aug_stat[b % 2]
                amv = aug_mov[b % 2]
                # xy x-part was pre-scaled by -2 on host, so sq sums give
                # 4*||x||^2; rescale by 0.25 on the PSUM->SBUF copy.
                a_sb = work.tile([1, NV], f32r, tag="asb", name=f"asb{b}")
                nc.scalar.mul(out=a_sb, in_=a_ps, mul=0.25)
                nc.sync.dma_start(out=ast[1:2, :], in_=a_sb)
                nc.scalar.copy(out=amv[0:1, :], in_=b_ps)

            load_batch(0)
            norm_stage(0)

            for b in range(B):
                if b + 1 < B:
                    load_batch(b + 1)
                xytile = xytiles[b]
                ast = aug_stat[b % 2]
                amv = aug_mov[b % 2]

                Rt = work.tile([P, NL], f32, tag="R")
                d1c = work.tile([P, MC], f32, tag="d1c")
                d2c = work.tile([P, KC], f32, tag="d2c")

                for m in range(MC):
                    pm = ps.tile([P, NL], f32, tag="P", bufs=3)
                    for k in range(KC):
                        nc.tensor.matmul(
                            out=pm,
                            lhsT=xytile[:, xsl(k, m * P, (m + 1) * P)],
                            rhs=xytile[:, xsl(k, NV, NT)],
                            start=(k == 0),
                            stop=False,
                        )
                    nc.tensor.matmul(
                        out=pm,
                        lhsT=ast[:, m * P : (m + 1) * P],
                        rhs=amv,
                        start=False,
                        stop=True,
                    )
                    # D1: min over l (free dim) for the 128 v of this chunk.
                    nc.vector.tensor_reduce(
                        out=d1c[:, m : m + 1], in_=pm, axis=AX.X, op=ALU.min
                    )
                    # D2: running elementwise min across v-chunks.
                    if m == 0:
                        nc.vector.tensor_copy(out=Rt, in_=pm)
                    else:
                        nc.vector.tensor_tensor(out=Rt, in0=Rt, in1=pm, op=ALU.min)

                # Pipelined: norms for the NEXT batch run here so this
                # batch's aug matmuls never wait on ACT at batch start.
                if b + 1 < B:
                    norm_stage(b + 1)

                # D2: reduce across the remaining 128 partitions via PE
                # transposes then free-dim mins.
                t_ps = ps.tile([P, NL], f32, tag="T", bufs=1)
                for j in range(KC):
                    nc.tensor.transpose(
                        out=t_ps[:, j * P : (j + 1) * P],
                        in_=Rt[:, j * P : (j + 1) * P],
                        identity=identity,
                    )
                for j in range(KC):
                    nc.vector.tensor_reduce(
                        out=d2c[:, j : j + 1],
                        in_=t_ps[:, j * P : (j + 1) * P],
                        axis=AX.X,
                        op=ALU.min,
                    )

                nc.vector.tensor_reduce(
                    out=d1sums[:, b : b + 1], in_=d1c, axis=AX.X, op=ALU.add
                )
                nc.vector.tensor_reduce(
                    out=d2sums[:, b : b + 1], in_=d2c, axis=AX.X, op=ALU.add
                )
                del xytiles[b]

            # out[b] = (sum_p d1sums + 2 * sum_p d2sums) / 1024
            nc.vector.scalar_tensor_tensor(
                out=dall,
                in0=d2sums,
                scalar=2.0,
                in1=d1sums,
                op0=ALU.mult,
                op1=ALU.add,
            )
            f_ps = psn.tile([1, B], f32, tag="fin")
            nc.tensor.matmul(
                out=f_ps, lhsT=ones_f32, rhs=dall, start=True, stop=True
            )
            nc.scalar.mul(out=out_sb, in_=f_ps, mul=1.0 / NV)
            nc.sync.dma_start(out=out_h[:], in_=out_sb)

    # Bacc defers register allocation + wait-splitting to finalize();
    # the pjrt execution path expects an already-finalized module.
    nc.finalize()
    return nc


def _get_bass():
    if "nc" not in _CACHE:
        _CACHE["nc"] = _build_bass()
    return _CACHE["nc"]


def _run(in_maps, trace=False):
    from concourse.bass_utils import run_bass_kernel_spmd

    nc = _get_bass()
    return run_bass_kernel_spmd(nc, in_maps, list(range(N_CORES)), trace=trace)


def round_fp32r(x):
    """Round f32 to fp32r (sign + 8 exp + 11 mantissa bits, RNE) — the
    precision the PE uses for float32r operands."""
    u = x.view(np.uint32)
    low = u & np.uint32(0xFFF)
    base = u & ~np.uint32(0xFFF)
    odd = ((base >> np.uint32(12)) & np.uint32(1)).astype(bool)
    round_up = (low > 0x800) | ((low == 0x800) & odd)
    out = base + (round_up.astype(np.uint32) << np.uint32(12))
    return out.view(np.float32)


def make_in_maps(video_feat, lang_feat):
    video = np.asarray(video_feat, dtype=np.float32)
    lang = np.asarray(lang_feat, dtype=np.float32)
    assert video.shape == (N_CORES * B, NV, D), video.shape
    assert lang.shape == (N_CORES * B, NL, D), lang.shape
    in_maps = []
    for c in range(N_CORES):
        vb = video[c * B : (c + 1) * B]
        lb = lang[c * B : (c + 1) * B]
        xy = np.empty((B, D, NT), np.float32)
        np.multiply(np.transpose(vb, (0, 2, 1)), np.float32(-2.0), out=xy[:, :, :NV])
        xy[:, :, NV:] = np.transpose(lb, (0, 2, 1))
        in_maps.append({"xy": round_fp32r(xy)})
    return in_maps


def kernel(video_feat, lang_feat):
    res = _run(make_in_maps(video_feat, lang_feat), trace=False)
    outs = [res.results[c]["out"].reshape(-1) for c in range(N_CORES)]
    return np.concatenate(outs).astype(np.float32)


# revision 27
# speedup vs baseline: 1.0896x; 1.0896x over previous
"""Chamfer distance kernel for Trainium2 (Bass/Tile), SPMD over 8 NeuronCores.

Math (per batch b):
  dist[v,l] = ||x_v||^2 - 2 x_v.y_l + ||y_l||^2,  x=[1024,512], y=[512,512]
  out[b] = mean_v min_l dist + mean_l min_v dist

Strategy:
  - Data-parallel over batch: 64 batches -> 8 cores x 8 batches.
  - Host passes one tensor xy[b] = concat(-2*x^T, y^T) along the row dim
    ([D, Nv+Nl] per batch) so both matmul operands have the contraction
    dim D on partitions, loaded with one large DMA per batch.
  - Per batch on-chip (all matmuls in fp32r: fp32 storage, fast PE path):
      squares (one ACT pass) -> column-of-ones matmuls (PE) give
        a_row = 4*||x||^2 (rescaled 0.25 on the PSUM->SBUF copy) and
        b_row = ||y||^2 as [1, N] row vectors.
      main matmul (PE): pm[v-chunk] = sum_k (-2x)^T_k . y^T_k, plus one
        K=2 augmentation matmul adding a_v + b_l broadcast via
        stationary [ones; a] x moving [b; ones] => pm = full dist chunk.
      D1: free-dim min per chunk (DVE) -> [128,1] columns -> summed.
      D2: running elementwise min across chunks (DVE), then 4 PE
        transposes + free-dim mins to reduce across partitions.
  - Final: cross-partition sums via ones-matmul (exact fp32), scale,
    DMA out [1, 8] per core; host concatenates to [64].
"""

import numpy as np

N_CORES = 8
B = 8          # batches per core
D = 512        # feature dim
NV = 1024      # video clips
NL = 512       # language tokens
P = 128        # partitions
KC = D // P    # contraction chunks = 4
MC = NV // P   # v chunks = 8
NT = NV + NL   # combined x|y row length per k-chunk

IO_BUFS = 4

_CACHE = {}


def _build_bass():
    import concourse.bass as bass
    import concourse.mybir as mybir
    import concourse.tile as tile
    from concourse import bacc
    from concourse.masks import make_identity

    f32 = mybir.dt.float32
    f32r = mybir.dt.float32r
    ALU = mybir.AluOpType
    AX = mybir.AxisListType
    SQUARE = mybir.ActivationFunctionType.Square

    nc = bacc.Bacc(None)
    xy_h = nc.declare_dram_parameter("xy", [B, D, NT], f32r, isOutput=False)
    out_h = nc.declare_dram_parameter("out", [1, B], f32, isOutput=True)

    with tile.TileContext(nc) as tc:
        with (
            tc.tile_pool(name="const", bufs=1) as cpool,
            tc.tile_pool(name="io", bufs=IO_BUFS) as io,
            tc.tile_pool(name="work", bufs=2) as work,
            tc.tile_pool(name="acc", bufs=1) as accp,
            tc.tile_pool(name="ps", bufs=2, space="PSUM") as ps,
            tc.tile_pool(name="psn", bufs=1, space="PSUM") as psn,
        ):
            identity = cpool.tile([P, P], f32, tag="ident")
            make_identity(nc, identity)
            ones_f32 = cpool.tile([P, 1], f32, tag="onesf")
            nc.vector.memset(ones_f32, 1.0)
            # Memset can't write float32r (ISA check); produce f32r ones
            # via ACT copies, which round on write.
            ones_col = cpool.tile([P, 1], f32r, tag="ones")
            nc.scalar.copy(out=ones_col, in_=ones_f32)
            ones2_f32 = cpool.tile([2, NV], f32, tag="ones2f")
            nc.vector.memset(ones2_f32, 1.0)

            # Augmentation operands (double buffered by batch parity):
            #   aug_stat: partition 0 = ones, partition 1 = a_row
            #   aug_mov:  partition 0 = b_row, partition 1 = ones
            # K=2 contraction pairs 1*b_n + a_m*1. Engine writes must start
            # at a 32-aligned partition: ones rows come from the one-time
            # whole-tile copies, per-batch b lands on partition 0 via ACT,
            # per-batch a reaches partition 1 via a SBUF->SBUF DMA.
            aug_stat = [
                cpool.tile([2, NV], f32r, tag=f"augs{i}", name=f"aug_stat{i}")
                for i in range(2)
            ]
            aug_mov = [
                cpool.tile([2, NL], f32r, tag=f"augm{i}", name=f"aug_mov{i}")
                for i in range(2)
            ]
            for t in aug_stat:
                nc.scalar.copy(out=t, in_=ones2_f32)
            for t in aug_mov:
                nc.scalar.copy(out=t, in_=ones2_f32[:, :NL])

            d1sums = accp.tile([P, B], f32, tag="d1s")
            d2sums = accp.tile([P, B], f32, tag="d2s")
            dall = accp.tile([P, B], f32, tag="dall")
            out_sb = accp.tile([1, B], f32, tag="osb")

            def xsl(k, lo, hi):
                return slice(k * NT + lo, k * NT + hi)

            for b in range(B):
                xytile = io.tile([P, KC * NT], f32r, tag="xy")
                nc.sync.dma_start(
                    out=xytile[:, : 2 * NT],
                    in_=xy_h[b, : 2 * P].rearrange("(k p) n -> p k n", p=P),
                )
                nc.sync.dma_start(
                    out=xytile[:, 2 * NT :],
                    in_=xy_h[b, 2 * P :].rearrange("(k p) n -> p k n", p=P),
                )

                # Squared elements for the norms (two ACT passes, one
                # per DMA half), then pairwise k-chunk adds on the
                # otherwise-idle GPSIMD to halve the norm matmul count.
                sq = work.tile([P, KC * NT], f32r, tag="sq")
                nc.scalar.activation(
                    out=sq[:, : 2 * NT], in_=xytile[:, : 2 * NT], func=SQUARE
                )
                nc.scalar.activation(
                    out=sq[:, 2 * NT :], in_=xytile[:, 2 * NT :], func=SQUARE
                )
                sqh = work.tile([P, 2 * NT], f32r, tag="sqh")
                nc.gpsimd.tensor_tensor(
                    out=sqh[:, :NT], in0=sq[:, :NT], in1=sq[:, NT : 2 * NT],
                    op=ALU.add,
                )
                nc.gpsimd.tensor_tensor(
                    out=sqh[:, NT:], in0=sq[:, 2 * NT : 3 * NT],
                    in1=sq[:, 3 * NT :], op=ALU.add,
                )

                # Cross-partition (over d) sums via ones-column matmuls.
                a_ps = psn.tile([1, NV], f32, tag="aps")
                b_ps = psn.tile([1, NL], f32, tag="bps")
                for k in range(2):
                    for h in range(2):
                        nc.tensor.matmul(
                            out=a_ps[:, h * 512 : (h + 1) * 512],
                            lhsT=ones_col,
                            rhs=sqh[:, k * NT + h * 512 : k * NT + (h + 1) * 512],
                            start=(k == 0),
                            stop=(k == 1),
                        )
                    nc.tensor.matmul(
                        out=b_ps,
                        lhsT=ones_col,
                        rhs=sqh[:, k * NT + NV : (k + 1) * NT],
                        start=(k == 0),
                        stop=(k == 1),
                    )

                ast = aug_stat[b % 2]
                amv = aug_mov[b % 2]
                # xy x-part was pre-scaled by -2 on host, so sq sums give
                # 4*||x||^2; rescale by 0.25 on the PSUM->SBUF copy.
                a_sb = work.tile([1, NV], f32r, tag="asb")
                nc.scalar.mul(out=a_sb, in_=a_ps, mul=0.25)
                nc.sync.dma_start(out=ast[1:2, :], in_=a_sb)
                nc.scalar.copy(out=amv[0:1, :], in_=b_ps)

                Rt = work.tile([P, NL], f32, tag="R")
                d1c = work.tile([P, MC], f32, tag="d1c")
                d2c = work.tile([P, KC], f32, tag="d2c")

                for m in range(MC):
                    pm = ps.tile([P, NL], f32, tag="P", bufs=3)
                    for k in range(KC):
                        nc.tensor.matmul(
                            out=pm,
                            lhsT=xytile[:, xsl(k, m * P, (m + 1) * P)],
                            rhs=xytile[:, xsl(k, NV, NT)],
                            start=(k == 0),
                            stop=False,
                        )
                    nc.tensor.matmul(
                        out=pm,
                        lhsT=ast[:, m * P : (m + 1) * P],
                        rhs=amv,
                        start=False,
                        stop=True,
                    )
                    # D1: min over l (free dim) for the 128 v of this chunk.
                    nc.vector.tensor_reduce(
                        out=d1c[:, m : m + 1], in_=pm, axis=AX.X, op=ALU.min
                    )
                    # D2: running elementwise min across v-chunks.
                    if m == 0:
                        nc.vector.tensor_copy(out=Rt, in_=pm)
                    else:
                        nc.vector.tensor_tensor(out=Rt, in0=Rt, in1=pm, op=ALU.min)

                # D2: reduce across the remaining 128 partitions via PE
                # transposes then free-dim mins.
                t_ps = ps.tile([P, NL], f32, tag="T", bufs=1)
                for j in range(KC):
                    nc.tensor.transpose(
                        out=t_ps[:, j * P : (j + 1) * P],
                        in_=Rt[:, j * P : (j + 1) * P],
                        identity=identity,
                    )
                for j in range(KC):
                    nc.vector.tensor_reduce(
                        out=d2c[:, j : j + 1],
                        in_=t_ps[:, j * P : (j + 1) * P],
                        axis=AX.X,
                        op=ALU.min,
                    )

                nc.vector.tensor_reduce(
                    out=d1sums[:, b : b + 1], in_=d1c, axis=AX.X, op=ALU.add
                )
                nc.vector.tensor_reduce(
                    out=d2sums[:, b : b + 1], in_=d2c, axis=AX.X, op=ALU.add
                )

            # out[b] = (sum_p d1sums + 2 * sum_p d2sums) / 1024
            nc.vector.scalar_tensor_tensor(
                out=dall,
                in0=d2sums,
                scalar=2.0,
                in1=d1sums,
                op0=ALU.mult,
                op1=ALU.add,
            )
            f_ps = psn.tile([1, B], f32, tag="fin")
            nc.tensor.matmul(
                out=f_ps, lhsT=ones_f32, rhs=dall, start=True, stop=True
            )
            nc.scalar.mul(out=out_sb, in_=f_ps, mul=1.0 / NV)
            nc.sync.dma_start(out=out_h[:], in_=out_sb)

    # Bacc defers register allocation + wait-splitting to finalize();
    # the pjrt execution path expects an already-finalized module.
    nc.finalize()
    return nc


def _get_bass():
    if "nc" not in _CACHE:
        _CACHE["nc"] = _build_bass()
    return _CACHE["nc"]


def _run(in_maps, trace=False):
    from concourse.bass_utils import run_bass_kernel_spmd

    nc = _get_bass()
    return run_bass_kernel_spmd(nc, in_maps, list(range(N_CORES)), trace=trace)


def round_fp32r(x):
    """Round f32 to fp32r (sign + 8 exp + 11 mantissa bits, RNE) — the
    precision the PE uses for float32r operands."""
    u = x.view(np.uint32)
    low = u & np.uint32(0xFFF)
    base = u & ~np.uint32(0xFFF)
    odd = ((base >> np.uint32(12)) & np.uint32(1)).astype(bool)
    round_up = (low > 0x800) | ((low == 0x800) & odd)
    out = base + (round_up.astype(np.uint32) << np.uint32(12))
    return out.view(np.float32)


def make_in_maps(video_feat, lang_feat):
    video = np.asarray(video_feat, dtype=np.float32)
    lang = np.asarray(lang_feat, dtype=np.float32)
    assert video.shape == (N_CORES * B, NV, D), video.shape
    assert lang.shape == (N_CORES * B, NL, D), lang.shape
    in_maps = []
    for c in range(N_CORES):
        vb = video[c * B : (c + 1) * B]
        lb = lang[c * B : (c + 1) * B]
        xy = np.empty((B, D, NT), np.float32)
        np.multiply(np.transpose(vb, (0, 2, 1)), np.float32(-2.0), out=xy[:, :, :NV])
        xy[:, :, NV:] = np.transpose(lb, (0, 2, 1))
        in_maps.append({"xy": round_fp32r(xy)})
    return in_maps


def kernel(video_feat, lang_feat):
    res = _run(make_in_maps(video_feat, lang_feat), trace=False)
    outs = [res.results[c]["out"].reshape(-1) for c in range(N_CORES)]
    return np.concatenate(outs).astype(np.float32)
